# revision 14
# baseline (speedup 1.0000x reference)
"""Causal self-attention (B=2, S=4096, D=512, H=8) on 8 Trainium2 cores.

Sharding: core c handles batch b = c//4 and heads {2*(c%4), 2*(c%4)+1}.
Each core computes q/k/v projections for its two heads, causal flash-style
attention in a transposed (k-major) score layout, and per-head undivided
output-projection partials po_h^T = Wo_h @ attn_h^T plus the softmax
denominators.  The host divides by the denominators, sums the 4 cores per
batch, adds bo, and transposes back.

Device layout notes:
  qT/kT: [128, S] bf16, rows 0-63 head0, 64-127 head1 (head dim on
  partitions).  scores^T tiles: [128 keys, 1024 queries]; exp on ACT reads
  PSUM directly with the padding mask folded into the per-partition bias
  and 1/sqrt(hd) into the scale, writing bf16.  V is PE-transposed to
  k-major [128, 65]-blocks with a ones column appended, so the PV matmul
  accumulates numerators and the softmax denominator (row 64) together.
  All matmul operands are bf16 (full-rate PE + fast weight load);
  accumulation stays fp32 in PSUM.
"""

import sys

sys.path.insert(0, "/opt/trn_rl_repo")

from contextlib import ExitStack

import ml_dtypes
import numpy as np

import concourse.bass as bass
import concourse.tile as tile
from concourse import bacc, bass_utils, mybir

B, S, D = 2, 4096, 512
H, HD = 8, 64
NCORES = 8
F32 = mybir.dt.float32
BF16 = mybir.dt.bfloat16
EXP = mybir.ActivationFunctionType.Exp
NPBF16 = ml_dtypes.bfloat16

CHUNK = 1024                  # query-chunk width
NCHUNK = S // CHUNK           # 4
KBLK = 128                    # key block (partition dim)
KB_PER_CHUNK = CHUNK // KBLK  # 8
NEG = -1.0e30


def _pieces(col0):
    """Split [col0, CHUNK) into <=512-wide pieces aligned to 512 boundaries."""
    out = []
    c = col0
    while c < CHUNK:
        nxt = min(CHUNK, (c // 512 + 1) * 512)
        out.append((c, nxt))
        c = nxt
    return out


def _emit(nc, tc, ctx, io):
    xT, wq_p, wk_p, wv_p, wo01d, bqkv, kbias, trimask, ident2, \
        po0T, po1T, dens = io

    const = ctx.enter_context(tc.tile_pool(name="const", bufs=1))
    poolA = ctx.enter_context(tc.tile_pool(name="poolA", bufs=1))
    poolB = ctx.enter_context(tc.tile_pool(name="poolB", bufs=1))

    # ---- constants / weights into SBUF ----
    wq_sb = const.tile([128, 512], BF16, tag="wq")
    wk_sb = const.tile([128, 512], BF16, tag="wk")
    wv_sb = const.tile([128, 512], BF16, tag="wv")
    wo01_sb = const.tile([128, 512], BF16, tag="wo01")
    bqkv_sb = const.tile([128, 3], F32, tag="bqkv")
    kbias_sb = const.tile([128, 32], F32, tag="kbias")
    tri_sb = const.tile([128, 128], BF16, tag="tri")
    id2_sb = const.tile([128, 64], BF16, tag="id2")
    onesf_sb = const.tile([128, 1], F32, tag="onesf")
    nc.vector.memset(onesf_sb[:], 1.0)
    for t, a in ((wq_sb, wq_p), (wk_sb, wk_p), (wv_sb, wv_p),
                 (bqkv_sb, bqkv), (id2_sb, ident2), (kbias_sb, kbias),
                 (tri_sb, trimask), (wo01_sb, wo01d)):
        nc.sync.dma_start(t[:], a[:])

    # ---- intermediates: poolA spans phases 1-2, poolB phases 2-3 ----
    qT = poolA.tile([128, S], BF16, tag="qT")
    kT = poolA.tile([128, S], BF16, tag="kT")
    v0 = poolA.tile([128, 32 * 65], BF16, tag="v0")
    v1 = poolA.tile([128, 32 * 65], BF16, tag="v1")
    oT01 = poolB.tile([128, S], BF16, tag="oT01")
    den0 = poolB.tile([1, S], F32, tag="den0")
    den1 = poolB.tile([1, S], F32, tag="den1")

    # ---- phase 1: q/k/v projections (+ v transpose to k-major) ----
    with tc.tile_pool(name="ph1sb", bufs=1) as p1s, \
         tc.tile_pool(name="ph1ps", bufs=2, space="PSUM") as p1p:
        vT = p1s.tile([128, S], BF16, tag="vT")

        for J in range(NCHUNK):
            x_sb = []
            for ks in range(4):
                xt = p1s.tile([128, CHUNK], BF16, tag=f"x{ks}", bufs=2)
                nc.sync.dma_start(
                    xt[:],
                    xT[ks * 128:(ks + 1) * 128, J * CHUNK:(J + 1) * CHUNK])
                x_sb.append(xt)
            for w_sb, bcol, dest in ((wq_sb, 0, qT), (wk_sb, 1, kT),
                                     (wv_sb, 2, vT)):
                ps = p1p.tile([128, CHUNK], F32, tag="proj")
                for half in range(2):
                    lo = half * 512
                    for ks in range(4):
                        nc.tensor.matmul(
                            ps[:, half * 512:(half + 1) * 512],
                            w_sb[:, ks * 128:(ks + 1) * 128],
                            x_sb[ks][:, lo:lo + 512],
                            start=(ks == 0), stop=(ks == 3),
                        )
                nc.vector.tensor_scalar_add(
                    dest[:, J * CHUNK:(J + 1) * CHUNK], ps[:],
                    bqkv_sb[:, bcol:bcol + 1])

        # V -> k-major blocks; head0/head1 transposes adjacent so they run
        # concurrently on PE row-groups 0-63 / 64-127
        for g in range(4):  # groups of 8 key-blocks
            tr0 = p1p.tile([128, 512], BF16, tag="vtr0")
            tr1 = p1p.tile([128, 512], BF16, tag="vtr1")
            trs = {0: tr0, 1: tr1}
            for i in range(8):
                kb = g * 8 + i
                for hh in (0, 1):
                    nc.tensor.transpose(
                        trs[hh][:, i * 64:(i + 1) * 64],
                        vT[hh * 64:(hh + 1) * 64, kb * KBLK:(kb + 1) * KBLK],
                        id2_sb[hh * 64:(hh + 1) * 64, :],
                    )
            for hh, vdst in ((0, v0), (1, v1)):
                dst = vdst[:, g * 8 * 65:(g + 1) * 8 * 65]
                dst = dst.rearrange("p (k c) -> p k c", c=65)[:, :, 0:64]
                nc.vector.tensor_copy(
                    dst, trs[hh].rearrange("p (k c) -> p k c", c=64))
        for vdst in (v0, v1):
            ones_col = vdst.rearrange("p (k c) -> p k c", c=65)[:, :, 64:65]
            nc.vector.tensor_copy(
                ones_col, onesf_sb[:].to_broadcast((128, 32, 1)))

    # ---- phase 2: attention, heads interleaved so the K=64 QK matmuls of
    # head0/head1 run concurrently on PE row-groups 0-63 / 64-127 ----
    with tc.tile_pool(name="etp", bufs=8) as etp, \
         tc.tile_pool(name="ps_st", bufs=1, space="PSUM") as ps_st, \
         tc.tile_pool(name="ps_pv", bufs=1, space="PSUM") as ps_pv:
        for J in reversed(range(NCHUNK)):
            pv0 = ps_pv.tile([65, CHUNK], F32, tag="pv0")
            pv1 = ps_pv.tile([65, CHUNK], F32, tag="pv1")
            nkb = KB_PER_CHUNK * (J + 1)
            for kb in range(nkb):
                p = kb - KB_PER_CHUNK * J
                col0 = KBLK * p if p >= 0 else 0
                pieces = _pieces(col0)
                st0 = ps_st.tile([128, CHUNK], F32, tag="st0")
                st1 = ps_st.tile([128, CHUNK], F32, tag="st1")
                for st, hh in ((st0, 0), (st1, 1)):
                    hsl = slice(hh * 64, (hh + 1) * 64)
                    for (a, b) in pieces:
                        nc.tensor.matmul(
                            st[:, a:b],
                            kT[hsl, kb * KBLK:(kb + 1) * KBLK],
                            qT[hsl, J * CHUNK + a:J * CHUNK + b],
                            start=True, stop=True,
                        )
                ets = []
                for st in (st0, st1):
                    et = etp.tile([128, CHUNK], BF16, tag="et")
                    nc.scalar.activation(
                        et[:, col0:], st[:, col0:], EXP,
                        bias=kbias_sb[:, kb:kb + 1], scale=0.125)
                    if p >= 0:
                        nc.vector.tensor_mul(
                            et[:, col0:col0 + KBLK], et[:, col0:col0 + KBLK],
                            tri_sb[:])
                    ets.append(et)
                for et, vsb, pv in ((ets[0], v0, pv0), (ets[1], v1, pv1)):
                    for (a, b) in pieces:
                        # stop exactly on the last matmul touching each
                        # 512-wide psum bank region
                        last_a = (kb == KB_PER_CHUNK * J + 3 and a < 512)
                        last_b = (kb == nkb - 1)
                        nc.tensor.matmul(
                            pv[:, a:b],
                            vsb[:, kb * 65:(kb + 1) * 65],
                            et[:, a:b],
                            start=(kb == 0),
                            stop=(last_a if a < 512 else last_b),
                        )
            csl = slice(J * CHUNK, (J + 1) * CHUNK)
            nc.vector.tensor_copy(oT01[0:64, csl], pv0[0:64, :])
            nc.vector.tensor_copy(oT01[64:128, csl], pv1[0:64, :])
            nc.vector.tensor_copy(den0[:, csl], pv0[64:65, :])
            nc.vector.tensor_copy(den1[:, csl], pv1[64:65, :])

    # ---- phase 3: per-head output projection (undivided), heads row-paired ----
    nc.sync.dma_start(dens[0:1, :], den0[:])
    nc.sync.dma_start(dens[1:2, :], den1[:])
    with tc.tile_pool(name="ph3sb", bufs=2) as p3s, \
         tc.tile_pool(name="ps_po", bufs=2, space="PSUM") as ps_po:
        for J in range(NCHUNK):
            csl = slice(J * CHUNK, (J + 1) * CHUNK)
            for dt_ in range(4):
                po0 = ps_po.tile([128, CHUNK], F32, tag="po0")
                po1 = ps_po.tile([128, CHUNK], F32, tag="po1")
                for (a, b) in _pieces(0):
                    for po, hh in ((po0, 0), (po1, 1)):
                        hsl = slice(hh * 64, (hh + 1) * 64)
                        nc.tensor.matmul(
                            po[:, a:b],
                            wo01_sb[hsl, dt_ * 128:(dt_ + 1) * 128],
                            oT01[hsl, J * CHUNK + a:J * CHUNK + b],
                            start=True, stop=True)
                for po, poT, eng in ((po0, po0T, nc.vector.tensor_copy),
                                     (po1, po1T, nc.scalar.copy)):
                    posb = p3s.tile([128, CHUNK], F32, tag="posb")
                    eng(posb[:], po[:])
                    nc.sync.dma_start(poT[dt_ * 128:(dt_ + 1) * 128, csl],
                                      posb[:])


_CACHED = None


def _build():
    global _CACHED
    if _CACHED is not None:
        return _CACHED
    nc = bacc.Bacc("TRN2", target_bir_lowering=False, debug=False,
                   enable_asserts=False, num_devices=NCORES)
    names = [
        ("xT", [D, S], BF16), ("wq_p", [128, 512], BF16),
        ("wk_p", [128, 512], BF16), ("wv_p", [128, 512], BF16),
        ("wo01", [128, 512], BF16),
        ("bqkv", [128, 3], F32), ("kbias", [128, 32], F32),
        ("trimask", [128, 128], BF16), ("ident2", [128, 64], BF16),
    ]
    aps = [nc.dram_tensor(n, sh, dt_, kind="ExternalInput").ap()
           for n, sh, dt_ in names]
    po0T = nc.dram_tensor("po0T", [D, S], F32, kind="ExternalOutput").ap()
    po1T = nc.dram_tensor("po1T", [D, S], F32, kind="ExternalOutput").ap()
    dens = nc.dram_tensor("dens", [2, S], F32, kind="ExternalOutput").ap()
    with tile.TileContext(nc) as tc, ExitStack() as ctx:
        _emit(nc, tc, ctx, aps + [po0T, po1T, dens])
    nc.compile()
    _CACHED = nc
    return nc


def _host_inputs(x, attention_mask, Wq, bq, Wk, bk, Wv, bv, Wo, bo):
    f = np.float32
    x = np.asarray(x, f)
    mask = np.asarray(attention_mask)
    Wq, Wk, Wv, Wo = (np.asarray(w, f) for w in (Wq, Wk, Wv, Wo))
    bq, bk, bv = (np.asarray(b_, f) for b_ in (bq, bk, bv))
    tri = np.triu(np.ones((128, 128), NPBF16))      # [k,q]: 1 where q >= k
    id2 = np.tile(np.eye(64, dtype=NPBF16), (2, 1))
    in_maps = []
    for c in range(NCORES):
        b = c // 4
        h0 = 2 * (c % 4)
        hsl = slice(64 * h0, 64 * h0 + 128)

        def pack_w(W):
            wt = W[hsl, :].T                        # [512, 128] = Wh^T
            return np.ascontiguousarray(
                wt.reshape(4, 128, 128).transpose(1, 0, 2)
                .reshape(128, 512).astype(NPBF16))

        wo_t = Wo[:, hsl].T.astype(NPBF16)           # [128, 512]
        kb = np.where(mask[b] != 0, f(0.0), f(NEG)).astype(f)
        in_maps.append({
            "xT": np.ascontiguousarray(x[b].T.astype(NPBF16)),
            "wq_p": pack_w(Wq), "wk_p": pack_w(Wk), "wv_p": pack_w(Wv),
            "wo01": np.ascontiguousarray(wo_t),
            "bqkv": np.ascontiguousarray(
                np.stack([bq[hsl], bk[hsl], bv[hsl]], axis=1)),
            "kbias": np.ascontiguousarray(kb.reshape(32, 128).T),
            "trimask": tri, "ident2": id2,
        })
    return in_maps


def _assemble(results, bo):
    out = np.zeros((B, S, D), np.float32)
    for c in range(NCORES):
        r = results[c]
        dens = r["dens"]
        part = r["po0T"] / dens[0:1, :] + r["po1T"] / dens[1:2, :]
        out[c // 4] += part.T
    out += np.asarray(bo, np.float32)
    return out


def kernel(**inputs) -> np.ndarray:
    nc = _build()
    in_maps = _host_inputs(**inputs)
    last_err = None
    for attempt in range(3):
        try:
            res = bass_utils.run_bass_kernel_spmd(
                nc, in_maps, core_ids=list(range(NCORES)))
            out = _assemble(res.results, inputs["bo"])
        except Exception as e:  # transient NRT/axon device errors
            last_err = e
            continue
        if np.isfinite(out).all():
            return out
        last_err = RuntimeError("non-finite output")
    raise last_err


def run_traced(inputs, **kwargs):
    """test.py helper: run with NTFF tracing, return (out, BassKernelResults)."""
    nc = _build()
    in_maps = _host_inputs(**inputs)
    res = bass_utils.run_bass_kernel_spmd(
        nc, in_maps, core_ids=list(range(NCORES)), trace=True, **kwargs)
    return _assemble(res.results, inputs["bo"]), res


# revision 15
# speedup vs baseline: 1.0737x; 1.0737x over previous
"""Causal self-attention (B=2, S=4096, D=512, H=8) on 8 Trainium2 cores.

Sharding: core c handles batch b = c//4 and heads {2*(c%4), 2*(c%4)+1}.
Each core computes q/k/v projections for its two heads, causal flash-style
attention in a transposed (k-major) score layout, and per-head undivided
output-projection partials po_h^T = Wo_h @ attn_h^T plus the softmax
denominators.  The host divides by the denominators, sums the 4 cores per
batch, adds bo, and transposes back.

Device layout notes:
  qT/kT: [128, S] bf16, rows 0-63 head0, 64-127 head1 (head dim on
  partitions).  scores^T tiles: [128 keys, 1024 queries]; exp on ACT reads
  PSUM directly with the padding mask folded into the per-partition bias
  and 1/sqrt(hd) into the scale, writing bf16.  V is PE-transposed to
  k-major [128, 65]-blocks with a ones column appended, so the PV matmul
  accumulates numerators and the softmax denominator (row 64) together.
  All matmul operands are bf16 (full-rate PE + fast weight load);
  accumulation stays fp32 in PSUM.
"""

import sys

sys.path.insert(0, "/opt/trn_rl_repo")

from contextlib import ExitStack

import ml_dtypes
import numpy as np

import concourse.bass as bass
import concourse.tile as tile
from concourse import bacc, bass_utils, mybir

B, S, D = 2, 4096, 512
H, HD = 8, 64
NCORES = 8
F32 = mybir.dt.float32
BF16 = mybir.dt.bfloat16
EXP = mybir.ActivationFunctionType.Exp
NPBF16 = ml_dtypes.bfloat16

CHUNK = 1024                  # query-chunk width
NCHUNK = S // CHUNK           # 4
KBLK = 128                    # key block (partition dim)
KB_PER_CHUNK = CHUNK // KBLK  # 8
NEG = -1.0e30


def _pieces(col0):
    """Split [col0, CHUNK) into <=512-wide pieces aligned to 512 boundaries."""
    out = []
    c = col0
    while c < CHUNK:
        nxt = min(CHUNK, (c // 512 + 1) * 512)
        out.append((c, nxt))
        c = nxt
    return out


def _emit(nc, tc, ctx, io):
    xT, wq_p, wk_p, wv_p, wo01d, bqkv, kbias, trimask, ident2, \
        po0T, po1T, dens = io

    const = ctx.enter_context(tc.tile_pool(name="const", bufs=1))
    poolA = ctx.enter_context(tc.tile_pool(name="poolA", bufs=1))
    poolB = ctx.enter_context(tc.tile_pool(name="poolB", bufs=1))

    # ---- constants / weights into SBUF ----
    wq_sb = const.tile([128, 512], BF16, tag="wq")
    wk_sb = const.tile([128, 512], BF16, tag="wk")
    wv_sb = const.tile([128, 512], BF16, tag="wv")
    wo01_sb = const.tile([128, 512], BF16, tag="wo01")
    bqkv_sb = const.tile([128, 3], F32, tag="bqkv")
    kbias_sb = const.tile([128, 32], F32, tag="kbias")
    tri_sb = const.tile([128, 128], BF16, tag="tri")
    id2_sb = const.tile([128, 64], BF16, tag="id2")
    onesf_sb = const.tile([128, 1], F32, tag="onesf")
    nc.vector.memset(onesf_sb[:], 1.0)
    for t, a in ((wq_sb, wq_p), (wk_sb, wk_p), (wv_sb, wv_p),
                 (wo01_sb, wo01d), (bqkv_sb, bqkv), (kbias_sb, kbias),
                 (tri_sb, trimask), (id2_sb, ident2)):
        nc.sync.dma_start(t[:], a[:])

    # ---- intermediates: poolA spans phases 1-2, poolB phases 2-3 ----
    qT = poolA.tile([128, S], BF16, tag="qT")
    kT = poolA.tile([128, S], BF16, tag="kT")
    v0 = poolA.tile([128, 32 * 65], BF16, tag="v0")
    v1 = poolA.tile([128, 32 * 65], BF16, tag="v1")
    oT01 = poolB.tile([128, S], BF16, tag="oT01")
    den0 = poolB.tile([1, S], F32, tag="den0")
    den1 = poolB.tile([1, S], F32, tag="den1")

    # ---- phase 1: q/k/v projections (+ v transpose to k-major) ----
    with tc.tile_pool(name="ph1sb", bufs=1) as p1s, \
         tc.tile_pool(name="ph1ps", bufs=2, space="PSUM") as p1p:
        vT = p1s.tile([128, S], BF16, tag="vT")

        for J in range(NCHUNK):
            x_sb = []
            for ks in range(4):
                xt = p1s.tile([128, CHUNK], BF16, tag=f"x{ks}", bufs=2)
                nc.sync.dma_start(
                    xt[:],
                    xT[ks * 128:(ks + 1) * 128, J * CHUNK:(J + 1) * CHUNK])
                x_sb.append(xt)
            for w_sb, bcol, dest in ((wq_sb, 0, qT), (wk_sb, 1, kT),
                                     (wv_sb, 2, vT)):
                ps = p1p.tile([128, CHUNK], F32, tag="proj")
                for half in range(2):
                    lo = half * 512
                    for ks in range(4):
                        nc.tensor.matmul(
                            ps[:, half * 512:(half + 1) * 512],
                            w_sb[:, ks * 128:(ks + 1) * 128],
                            x_sb[ks][:, lo:lo + 512],
                            start=(ks == 0), stop=(ks == 3),
                        )
                nc.vector.tensor_scalar_add(
                    dest[:, J * CHUNK:(J + 1) * CHUNK], ps[:],
                    bqkv_sb[:, bcol:bcol + 1])

        # V -> k-major blocks (8 per PE-transpose batch) with ones column
        for hh, vdst in ((0, v0), (1, v1)):
            for g in range(4):  # groups of 8 key-blocks
                tr = p1p.tile([128, 512], BF16, tag="vtr")
                for i in range(8):
                    kb = g * 8 + i
                    nc.tensor.transpose(
                        tr[:, i * 64:(i + 1) * 64],
                        vT[hh * 64:(hh + 1) * 64, kb * KBLK:(kb + 1) * KBLK],
                        id2_sb[hh * 64:(hh + 1) * 64, :],
                    )
                dst = vdst[:, g * 8 * 65:(g + 1) * 8 * 65]
                dst = dst.rearrange("p (k c) -> p k c", c=65)[:, :, 0:64]
                nc.vector.tensor_copy(
                    dst, tr.rearrange("p (k c) -> p k c", c=64))
            ones_col = vdst.rearrange("p (k c) -> p k c", c=65)[:, :, 64:65]
            nc.vector.tensor_copy(
                ones_col, onesf_sb[:].to_broadcast((128, 32, 1)))

    # ---- phase 2: attention, heads interleaved so the K=64 QK matmuls of
    # head0/head1 run concurrently on PE row-groups 0-63 / 64-127 ----
    with tc.tile_pool(name="etp", bufs=6) as etp, \
         tc.tile_pool(name="ps_st", bufs=1, space="PSUM") as ps_st, \
         tc.tile_pool(name="ps_pv", bufs=1, space="PSUM") as ps_pv:
        for J in range(NCHUNK):
            pv0 = ps_pv.tile([65, CHUNK], F32, tag="pv0")
            pv1 = ps_pv.tile([65, CHUNK], F32, tag="pv1")
            nkb = KB_PER_CHUNK * (J + 1)
            for kb in range(nkb):
                p = kb - KB_PER_CHUNK * J
                col0 = KBLK * p if p >= 0 else 0
                pieces = _pieces(col0)
                st0 = ps_st.tile([128, CHUNK], F32, tag="st0")
                st1 = ps_st.tile([128, CHUNK], F32, tag="st1")
                for st, hh in ((st0, 0), (st1, 1)):
                    hsl = slice(hh * 64, (hh + 1) * 64)
                    for (a, b) in pieces:
                        nc.tensor.matmul(
                            st[:, a:b],
                            kT[hsl, kb * KBLK:(kb + 1) * KBLK],
                            qT[hsl, J * CHUNK + a:J * CHUNK + b],
                            start=True, stop=True,
                        )
                ets = []
                for st in (st0, st1):
                    et = etp.tile([128, CHUNK], BF16, tag="et")
                    nc.scalar.activation(
                        et[:, col0:], st[:, col0:], EXP,
                        bias=kbias_sb[:, kb:kb + 1], scale=0.125)
                    if p >= 0:
                        nc.vector.tensor_mul(
                            et[:, col0:col0 + KBLK], et[:, col0:col0 + KBLK],
                            tri_sb[:])
                    ets.append(et)
                for et, vsb, pv in ((ets[0], v0, pv0), (ets[1], v1, pv1)):
                    for (a, b) in pieces:
                        # stop exactly on the last matmul touching each
                        # 512-wide psum bank region
                        last_a = (kb == KB_PER_CHUNK * J + 3 and a < 512)
                        last_b = (kb == nkb - 1)
                        nc.tensor.matmul(
                            pv[:, a:b],
                            vsb[:, kb * 65:(kb + 1) * 65],
                            et[:, a:b],
                            start=(kb == 0),
                            stop=(last_a if a < 512 else last_b),
                        )
            csl = slice(J * CHUNK, (J + 1) * CHUNK)
            nc.vector.tensor_copy(oT01[0:64, csl], pv0[0:64, :])
            nc.vector.tensor_copy(oT01[64:128, csl], pv1[0:64, :])
            nc.vector.tensor_copy(den0[:, csl], pv0[64:65, :])
            nc.vector.tensor_copy(den1[:, csl], pv1[64:65, :])

    # ---- phase 3: per-head output projection (undivided), heads row-paired ----
    nc.sync.dma_start(dens[0:1, :], den0[:])
    nc.sync.dma_start(dens[1:2, :], den1[:])
    with tc.tile_pool(name="ph3sb", bufs=2) as p3s, \
         tc.tile_pool(name="ps_po", bufs=2, space="PSUM") as ps_po:
        for J in range(NCHUNK):
            csl = slice(J * CHUNK, (J + 1) * CHUNK)
            for dt_ in range(4):
                po0 = ps_po.tile([128, CHUNK], F32, tag="po0")
                po1 = ps_po.tile([128, CHUNK], F32, tag="po1")
                for (a, b) in _pieces(0):
                    for po, hh in ((po0, 0), (po1, 1)):
                        hsl = slice(hh * 64, (hh + 1) * 64)
                        nc.tensor.matmul(
                            po[:, a:b],
                            wo01_sb[hsl, dt_ * 128:(dt_ + 1) * 128],
                            oT01[hsl, J * CHUNK + a:J * CHUNK + b],
                            start=True, stop=True)
                for po, poT in ((po0, po0T), (po1, po1T)):
                    posb = p3s.tile([128, CHUNK], F32, tag="posb")
                    nc.vector.tensor_copy(posb[:], po[:])
                    nc.sync.dma_start(poT[dt_ * 128:(dt_ + 1) * 128, csl],
                                      posb[:])


_CACHED = None


def _build():
    global _CACHED
    if _CACHED is not None:
        return _CACHED
    nc = bacc.Bacc("TRN2", target_bir_lowering=False, debug=False,
                   enable_asserts=False, num_devices=NCORES)
    names = [
        ("xT", [D, S], BF16), ("wq_p", [128, 512], BF16),
        ("wk_p", [128, 512], BF16), ("wv_p", [128, 512], BF16),
        ("wo01", [128, 512], BF16),
        ("bqkv", [128, 3], F32), ("kbias", [128, 32], F32),
        ("trimask", [128, 128], BF16), ("ident2", [128, 64], BF16),
    ]
    aps = [nc.dram_tensor(n, sh, dt_, kind="ExternalInput").ap()
           for n, sh, dt_ in names]
    po0T = nc.dram_tensor("po0T", [D, S], F32, kind="ExternalOutput").ap()
    po1T = nc.dram_tensor("po1T", [D, S], F32, kind="ExternalOutput").ap()
    dens = nc.dram_tensor("dens", [2, S], F32, kind="ExternalOutput").ap()
    with tile.TileContext(nc) as tc, ExitStack() as ctx:
        _emit(nc, tc, ctx, aps + [po0T, po1T, dens])
    nc.compile()
    _CACHED = nc
    return nc


def _host_inputs(x, attention_mask, Wq, bq, Wk, bk, Wv, bv, Wo, bo):
    f = np.float32
    x = np.asarray(x, f)
    mask = np.asarray(attention_mask)
    Wq, Wk, Wv, Wo = (np.asarray(w, f) for w in (Wq, Wk, Wv, Wo))
    bq, bk, bv = (np.asarray(b_, f) for b_ in (bq, bk, bv))
    tri = np.triu(np.ones((128, 128), NPBF16))      # [k,q]: 1 where q >= k
    id2 = np.tile(np.eye(64, dtype=NPBF16), (2, 1))
    in_maps = []
    for c in range(NCORES):
        b = c // 4
        h0 = 2 * (c % 4)
        hsl = slice(64 * h0, 64 * h0 + 128)

        def pack_w(W):
            wt = W[hsl, :].T                        # [512, 128] = Wh^T
            return np.ascontiguousarray(
                wt.reshape(4, 128, 128).transpose(1, 0, 2)
                .reshape(128, 512).astype(NPBF16))

        wo_t = Wo[:, hsl].T.astype(NPBF16)           # [128, 512]
        kb = np.where(mask[b] != 0, f(0.0), f(NEG)).astype(f)
        in_maps.append({
            "xT": np.ascontiguousarray(x[b].T.astype(NPBF16)),
            "wq_p": pack_w(Wq), "wk_p": pack_w(Wk), "wv_p": pack_w(Wv),
            "wo01": np.ascontiguousarray(wo_t),
            "bqkv": np.ascontiguousarray(
                np.stack([bq[hsl], bk[hsl], bv[hsl]], axis=1)),
            "kbias": np.ascontiguousarray(kb.reshape(32, 128).T),
            "trimask": tri, "ident2": id2,
        })
    return in_maps


def _assemble(results, bo):
    out = np.zeros((B, S, D), np.float32)
    for c in range(NCORES):
        r = results[c]
        dens = r["dens"]
        part = r["po0T"] / dens[0:1, :] + r["po1T"] / dens[1:2, :]
        out[c // 4] += part.T
    out += np.asarray(bo, np.float32)
    return out


def kernel(**inputs) -> np.ndarray:
    nc = _build()
    in_maps = _host_inputs(**inputs)
    last_err = None
    for attempt in range(3):
        try:
            res = bass_utils.run_bass_kernel_spmd(
                nc, in_maps, core_ids=list(range(NCORES)))
            out = _assemble(res.results, inputs["bo"])
        except Exception as e:  # transient NRT/axon device errors
            last_err = e
            continue
        if np.isfinite(out).all():
            return out
        last_err = RuntimeError("non-finite output")
    raise last_err


def run_traced(inputs, **kwargs):
    """test.py helper: run with NTFF tracing, return (out, BassKernelResults)."""
    nc = _build()
    in_maps = _host_inputs(**inputs)
    res = bass_utils.run_bass_kernel_spmd(
        nc, in_maps, core_ids=list(range(NCORES)), trace=True, **kwargs)
    return _assemble(res.results, inputs["bo"]), res


# revision 16
# speedup vs baseline: 1.1257x; 1.0485x over previous
"""Causal self-attention (B=2, S=4096, D=512, H=8) on 8 Trainium2 cores.

Sharding: core c handles batch b = c//4 and heads {2*(c%4), 2*(c%4)+1}.
Each core computes q/k/v projections for its two heads, causal flash-style
attention in a transposed (k-major) score layout, and per-head undivided
output-projection partials po_h^T = Wo_h @ attn_h^T plus the softmax
denominators.  The host divides by the denominators, sums the 4 cores per
batch, adds bo, and transposes back.

Device layout notes:
  qT/kT: [128, S] bf16, rows 0-63 head0, 64-127 head1 (head dim on
  partitions).  scores^T tiles: [128 keys, 1024 queries]; exp on ACT reads
  PSUM directly with the padding mask folded into the per-partition bias
  and 1/sqrt(hd) into the scale, writing bf16.  V is PE-transposed to
  k-major [128, 65]-blocks with a ones column appended, so the PV matmul
  accumulates numerators and the softmax denominator (row 64) together.
  All matmul operands are bf16 (full-rate PE + fast weight load);
  accumulation stays fp32 in PSUM.
"""

import sys

sys.path.insert(0, "/opt/trn_rl_repo")

from contextlib import ExitStack

import ml_dtypes
import numpy as np

import concourse.bass as bass
import concourse.tile as tile
from concourse import bacc, bass_utils, mybir

B, S, D = 2, 4096, 512
H, HD = 8, 64
NCORES = 8
F32 = mybir.dt.float32
BF16 = mybir.dt.bfloat16
EXP = mybir.ActivationFunctionType.Exp
NPBF16 = ml_dtypes.bfloat16

CHUNK = 1024                  # query-chunk width
NCHUNK = S // CHUNK           # 4
KBLK = 128                    # key block (partition dim)
KB_PER_CHUNK = CHUNK // KBLK  # 8
NEG = -1.0e30


def _pieces(col0):
    """Split [col0, CHUNK) into <=512-wide pieces aligned to 512 boundaries."""
    out = []
    c = col0
    while c < CHUNK:
        nxt = min(CHUNK, (c // 512 + 1) * 512)
        out.append((c, nxt))
        c = nxt
    return out


def _emit(nc, tc, ctx, io):
    xT, wq_p, wk_p, wv_p, wo01d, bqkv, kbias, trimask, ident2, \
        po0T, po1T, dens = io

    const = ctx.enter_context(tc.tile_pool(name="const", bufs=1))
    poolA = ctx.enter_context(tc.tile_pool(name="poolA", bufs=1))
    poolB = ctx.enter_context(tc.tile_pool(name="poolB", bufs=1))

    # ---- constants / weights into SBUF ----
    wq_sb = const.tile([128, 512], BF16, tag="wq")
    wk_sb = const.tile([128, 512], BF16, tag="wk")
    wv_sb = const.tile([128, 512], BF16, tag="wv")
    wo01_sb = const.tile([128, 512], BF16, tag="wo01")
    bqkv_sb = const.tile([128, 3], F32, tag="bqkv")
    kbias_sb = const.tile([128, 32], F32, tag="kbias")
    tri_sb = const.tile([128, 128], BF16, tag="tri")
    id2_sb = const.tile([128, 64], BF16, tag="id2")
    onesf_sb = const.tile([128, 1], F32, tag="onesf")
    nc.vector.memset(onesf_sb[:], 1.0)
    for t, a in ((wq_sb, wq_p), (wk_sb, wk_p), (wv_sb, wv_p),
                 (bqkv_sb, bqkv), (id2_sb, ident2), (kbias_sb, kbias),
                 (tri_sb, trimask), (wo01_sb, wo01d)):
        nc.sync.dma_start(t[:], a[:])

    # ---- intermediates: poolA spans phases 1-2, poolB phases 2-3 ----
    qT = poolA.tile([128, S], BF16, tag="qT")
    kT = poolA.tile([128, S], BF16, tag="kT")
    v0 = poolA.tile([128, 32 * 65], BF16, tag="v0")
    v1 = poolA.tile([128, 32 * 65], BF16, tag="v1")
    oT01 = poolB.tile([128, S], BF16, tag="oT01")
    den0 = poolB.tile([1, S], F32, tag="den0")
    den1 = poolB.tile([1, S], F32, tag="den1")

    # ---- phase 1: q/k/v projections (+ v transpose to k-major) ----
    with tc.tile_pool(name="ph1sb", bufs=1) as p1s, \
         tc.tile_pool(name="ph1ps", bufs=2, space="PSUM") as p1p:
        vT = p1s.tile([128, S], BF16, tag="vT")

        for J in range(NCHUNK):
            x_sb = []
            for ks in range(4):
                xt = p1s.tile([128, CHUNK], BF16, tag=f"x{ks}", bufs=2)
                nc.sync.dma_start(
                    xt[:],
                    xT[ks * 128:(ks + 1) * 128, J * CHUNK:(J + 1) * CHUNK])
                x_sb.append(xt)
            for w_sb, bcol, dest in ((wq_sb, 0, qT), (wk_sb, 1, kT),
                                     (wv_sb, 2, vT)):
                ps = p1p.tile([128, CHUNK], F32, tag="proj")
                for half in range(2):
                    lo = half * 512
                    for ks in range(4):
                        nc.tensor.matmul(
                            ps[:, half * 512:(half + 1) * 512],
                            w_sb[:, ks * 128:(ks + 1) * 128],
                            x_sb[ks][:, lo:lo + 512],
                            start=(ks == 0), stop=(ks == 3),
                        )
                nc.vector.tensor_scalar_add(
                    dest[:, J * CHUNK:(J + 1) * CHUNK], ps[:],
                    bqkv_sb[:, bcol:bcol + 1])

        # V -> k-major blocks (8 per PE-transpose batch) with ones column
        for hh, vdst in ((0, v0), (1, v1)):
            for g in range(4):  # groups of 8 key-blocks
                tr = p1p.tile([128, 512], BF16, tag="vtr")
                for i in range(8):
                    kb = g * 8 + i
                    nc.tensor.transpose(
                        tr[:, i * 64:(i + 1) * 64],
                        vT[hh * 64:(hh + 1) * 64, kb * KBLK:(kb + 1) * KBLK],
                        id2_sb[hh * 64:(hh + 1) * 64, :],
                    )
                dst = vdst[:, g * 8 * 65:(g + 1) * 8 * 65]
                dst = dst.rearrange("p (k c) -> p k c", c=65)[:, :, 0:64]
                nc.vector.tensor_copy(
                    dst, tr.rearrange("p (k c) -> p k c", c=64))
            ones_col = vdst.rearrange("p (k c) -> p k c", c=65)[:, :, 64:65]
            nc.vector.tensor_copy(
                ones_col, onesf_sb[:].to_broadcast((128, 32, 1)))

    # ---- phase 2: attention, heads interleaved so the K=64 QK matmuls of
    # head0/head1 run concurrently on PE row-groups 0-63 / 64-127 ----
    with tc.tile_pool(name="etp", bufs=6) as etp, \
         tc.tile_pool(name="ps_st", bufs=1, space="PSUM") as ps_st, \
         tc.tile_pool(name="ps_pv", bufs=1, space="PSUM") as ps_pv:
        for J in range(NCHUNK):
            pv0 = ps_pv.tile([65, CHUNK], F32, tag="pv0")
            pv1 = ps_pv.tile([65, CHUNK], F32, tag="pv1")
            nkb = KB_PER_CHUNK * (J + 1)
            for kb in range(nkb):
                p = kb - KB_PER_CHUNK * J
                col0 = KBLK * p if p >= 0 else 0
                pieces = _pieces(col0)
                st0 = ps_st.tile([128, CHUNK], F32, tag="st0")
                st1 = ps_st.tile([128, CHUNK], F32, tag="st1")
                for st, hh in ((st0, 0), (st1, 1)):
                    hsl = slice(hh * 64, (hh + 1) * 64)
                    for (a, b) in pieces:
                        nc.tensor.matmul(
                            st[:, a:b],
                            kT[hsl, kb * KBLK:(kb + 1) * KBLK],
                            qT[hsl, J * CHUNK + a:J * CHUNK + b],
                            start=True, stop=True,
                        )
                ets = []
                for st in (st0, st1):
                    et = etp.tile([128, CHUNK], BF16, tag="et")
                    nc.scalar.activation(
                        et[:, col0:], st[:, col0:], EXP,
                        bias=kbias_sb[:, kb:kb + 1], scale=0.125)
                    if p >= 0:
                        nc.vector.tensor_mul(
                            et[:, col0:col0 + KBLK], et[:, col0:col0 + KBLK],
                            tri_sb[:])
                    ets.append(et)
                for et, vsb, pv in ((ets[0], v0, pv0), (ets[1], v1, pv1)):
                    for (a, b) in pieces:
                        # stop exactly on the last matmul touching each
                        # 512-wide psum bank region
                        last_a = (kb == KB_PER_CHUNK * J + 3 and a < 512)
                        last_b = (kb == nkb - 1)
                        nc.tensor.matmul(
                            pv[:, a:b],
                            vsb[:, kb * 65:(kb + 1) * 65],
                            et[:, a:b],
                            start=(kb == 0),
                            stop=(last_a if a < 512 else last_b),
                        )
            csl = slice(J * CHUNK, (J + 1) * CHUNK)
            nc.vector.tensor_copy(oT01[0:64, csl], pv0[0:64, :])
            nc.vector.tensor_copy(oT01[64:128, csl], pv1[0:64, :])
            nc.vector.tensor_copy(den0[:, csl], pv0[64:65, :])
            nc.vector.tensor_copy(den1[:, csl], pv1[64:65, :])

    # ---- phase 3: per-head output projection (undivided), heads row-paired ----
    nc.sync.dma_start(dens[0:1, :], den0[:])
    nc.sync.dma_start(dens[1:2, :], den1[:])
    with tc.tile_pool(name="ph3sb", bufs=2) as p3s, \
         tc.tile_pool(name="ps_po", bufs=2, space="PSUM") as ps_po:
        for J in range(NCHUNK):
            csl = slice(J * CHUNK, (J + 1) * CHUNK)
            for dt_ in range(4):
                po0 = ps_po.tile([128, CHUNK], F32, tag="po0")
                po1 = ps_po.tile([128, CHUNK], F32, tag="po1")
                for (a, b) in _pieces(0):
                    for po, hh in ((po0, 0), (po1, 1)):
                        hsl = slice(hh * 64, (hh + 1) * 64)
                        nc.tensor.matmul(
                            po[:, a:b],
                            wo01_sb[hsl, dt_ * 128:(dt_ + 1) * 128],
                            oT01[hsl, J * CHUNK + a:J * CHUNK + b],
                            start=True, stop=True)
                for po, poT, cpy in ((po0, po0T, nc.vector.tensor_copy),
                                     (po1, po1T, nc.scalar.copy)):
                    posb = p3s.tile([128, CHUNK], F32, tag="posb", bufs=4)
                    cpy(posb[:], po[:])
                    nc.sync.dma_start(poT[dt_ * 128:(dt_ + 1) * 128, csl],
                                      posb[:])


_CACHED = None


def _build():
    global _CACHED
    if _CACHED is not None:
        return _CACHED
    nc = bacc.Bacc("TRN2", target_bir_lowering=False, debug=False,
                   enable_asserts=False, num_devices=NCORES)
    names = [
        ("xT", [D, S], BF16), ("wq_p", [128, 512], BF16),
        ("wk_p", [128, 512], BF16), ("wv_p", [128, 512], BF16),
        ("wo01", [128, 512], BF16),
        ("bqkv", [128, 3], F32), ("kbias", [128, 32], F32),
        ("trimask", [128, 128], BF16), ("ident2", [128, 64], BF16),
    ]
    aps = [nc.dram_tensor(n, sh, dt_, kind="ExternalInput").ap()
           for n, sh, dt_ in names]
    po0T = nc.dram_tensor("po0T", [D, S], F32, kind="ExternalOutput").ap()
    po1T = nc.dram_tensor("po1T", [D, S], F32, kind="ExternalOutput").ap()
    dens = nc.dram_tensor("dens", [2, S], F32, kind="ExternalOutput").ap()
    with tile.TileContext(nc) as tc, ExitStack() as ctx:
        _emit(nc, tc, ctx, aps + [po0T, po1T, dens])
    nc.compile()
    _CACHED = nc
    return nc


def _host_inputs(x, attention_mask, Wq, bq, Wk, bk, Wv, bv, Wo, bo):
    f = np.float32
    x = np.asarray(x, f)
    mask = np.asarray(attention_mask)
    Wq, Wk, Wv, Wo = (np.asarray(w, f) for w in (Wq, Wk, Wv, Wo))
    bq, bk, bv = (np.asarray(b_, f) for b_ in (bq, bk, bv))
    tri = np.triu(np.ones((128, 128), NPBF16))      # [k,q]: 1 where q >= k
    id2 = np.tile(np.eye(64, dtype=NPBF16), (2, 1))
    in_maps = []
    for c in range(NCORES):
        b = c // 4
        h0 = 2 * (c % 4)
        hsl = slice(64 * h0, 64 * h0 + 128)

        def pack_w(W):
            wt = W[hsl, :].T                        # [512, 128] = Wh^T
            return np.ascontiguousarray(
                wt.reshape(4, 128, 128).transpose(1, 0, 2)
                .reshape(128, 512).astype(NPBF16))

        wo_t = Wo[:, hsl].T.astype(NPBF16)           # [128, 512]
        kb = np.where(mask[b] != 0, f(0.0), f(NEG)).astype(f)
        in_maps.append({
            "xT": np.ascontiguousarray(x[b].T.astype(NPBF16)),
            "wq_p": pack_w(Wq), "wk_p": pack_w(Wk), "wv_p": pack_w(Wv),
            "wo01": np.ascontiguousarray(wo_t),
            "bqkv": np.ascontiguousarray(
                np.stack([bq[hsl], bk[hsl], bv[hsl]], axis=1)),
            "kbias": np.ascontiguousarray(kb.reshape(32, 128).T),
            "trimask": tri, "ident2": id2,
        })
    return in_maps


def _assemble(results, bo):
    out = np.zeros((B, S, D), np.float32)
    for c in range(NCORES):
        r = results[c]
        dens = r["dens"]
        part = r["po0T"] / dens[0:1, :] + r["po1T"] / dens[1:2, :]
        out[c // 4] += part.T
    out += np.asarray(bo, np.float32)
    return out


def kernel(**inputs) -> np.ndarray:
    nc = _build()
    in_maps = _host_inputs(**inputs)
    last_err = None
    for attempt in range(3):
        try:
            res = bass_utils.run_bass_kernel_spmd(
                nc, in_maps, core_ids=list(range(NCORES)))
            out = _assemble(res.results, inputs["bo"])
        except Exception as e:  # transient NRT/axon device errors
            last_err = e
            continue
        if np.isfinite(out).all():
            return out
        last_err = RuntimeError("non-finite output")
    raise last_err


def run_traced(inputs, **kwargs):
    """test.py helper: run with NTFF tracing, return (out, BassKernelResults)."""
    nc = _build()
    in_maps = _host_inputs(**inputs)
    res = bass_utils.run_bass_kernel_spmd(
        nc, in_maps, core_ids=list(range(NCORES)), trace=True, **kwargs)
    return _assemble(res.results, inputs["bo"]), res


# revision 17
# speedup vs baseline: 1.1924x; 1.0592x over previous
"""Causal self-attention (B=2, S=4096, D=512, H=8) on 8 Trainium2 cores.

Sharding: core c handles batch b = c//4 and heads {2*(c%4), 2*(c%4)+1}.
Each core computes q/k/v projections for its two heads, causal flash-style
attention in a transposed (k-major) score layout, and per-head undivided
output-projection partials po_h^T = Wo_h @ attn_h^T plus the softmax
denominators.  The host divides by the denominators, sums the 4 cores per
batch, adds bo, and transposes back.

Device layout notes:
  qT/kT: [128, S] bf16, rows 0-63 head0, 64-127 head1 (head dim on
  partitions).  scores^T tiles: [128 keys, 1024 queries]; exp on ACT reads
  PSUM directly with the padding mask folded into the per-partition bias
  and 1/sqrt(hd) into the scale, writing bf16.  V is PE-transposed to
  k-major [128, 65]-blocks with a ones column appended, so the PV matmul
  accumulates numerators and the softmax denominator (row 64) together.
  All matmul operands are bf16 (full-rate PE + fast weight load);
  accumulation stays fp32 in PSUM.
"""

import sys

sys.path.insert(0, "/opt/trn_rl_repo")

from contextlib import ExitStack

import ml_dtypes
import numpy as np

import concourse.bass as bass
import concourse.tile as tile
from concourse import bacc, bass_utils, mybir

B, S, D = 2, 4096, 512
H, HD = 8, 64
NCORES = 8
F32 = mybir.dt.float32
BF16 = mybir.dt.bfloat16
EXP = mybir.ActivationFunctionType.Exp
NPBF16 = ml_dtypes.bfloat16

CHUNK = 1024                  # query-chunk width
NCHUNK = S // CHUNK           # 4
KBLK = 128                    # key block (partition dim)
KB_PER_CHUNK = CHUNK // KBLK  # 8
NEG = -1.0e30


def _pieces(col0):
    """Split [col0, CHUNK) into <=512-wide pieces aligned to 512 boundaries."""
    out = []
    c = col0
    while c < CHUNK:
        nxt = min(CHUNK, (c // 512 + 1) * 512)
        out.append((c, nxt))
        c = nxt
    return out


def _emit(nc, tc, ctx, io):
    xT, wq_p, wk_p, wv_p, wo01d, bqkv, kbias, trimask, ident2, \
        po0T, po1T, dens = io

    const = ctx.enter_context(tc.tile_pool(name="const", bufs=1))
    poolA = ctx.enter_context(tc.tile_pool(name="poolA", bufs=1))
    poolB = ctx.enter_context(tc.tile_pool(name="poolB", bufs=1))

    # ---- constants / weights into SBUF ----
    wq_sb = const.tile([128, 512], BF16, tag="wq")
    wk_sb = const.tile([128, 512], BF16, tag="wk")
    wv_sb = const.tile([128, 512], BF16, tag="wv")
    wo01_sb = const.tile([128, 512], BF16, tag="wo01")
    bqkv_sb = const.tile([128, 3], F32, tag="bqkv")
    kbias_sb = const.tile([128, 32], F32, tag="kbias")
    tri_sb = const.tile([128, 128], BF16, tag="tri")
    id2_sb = const.tile([128, 64], BF16, tag="id2")
    onesf_sb = const.tile([128, 1], F32, tag="onesf")
    nc.vector.memset(onesf_sb[:], 1.0)
    for t, a in ((wq_sb, wq_p), (wk_sb, wk_p), (wv_sb, wv_p),
                 (bqkv_sb, bqkv), (id2_sb, ident2), (kbias_sb, kbias),
                 (tri_sb, trimask), (wo01_sb, wo01d)):
        nc.sync.dma_start(t[:], a[:])

    # ---- intermediates: poolA spans phases 1-2, poolB phases 2-3 ----
    qT = poolA.tile([128, S], BF16, tag="qT")
    kT = poolA.tile([128, S], BF16, tag="kT")
    v0 = poolA.tile([128, 32 * 65], BF16, tag="v0")
    v1 = poolA.tile([128, 32 * 65], BF16, tag="v1")
    oT01 = poolB.tile([128, S], BF16, tag="oT01")
    den0 = poolB.tile([1, S], F32, tag="den0")
    den1 = poolB.tile([1, S], F32, tag="den1")

    # ---- phase 1: q/k/v projections (+ v transpose to k-major) ----
    with tc.tile_pool(name="ph1sb", bufs=1) as p1s, \
         tc.tile_pool(name="ph1ps", bufs=2, space="PSUM") as p1p:
        vT = p1s.tile([128, S], BF16, tag="vT")

        for J in range(NCHUNK):
            x_sb = []
            for ks in range(4):
                xt = p1s.tile([128, CHUNK], BF16, tag=f"x{ks}", bufs=2)
                nc.sync.dma_start(
                    xt[:],
                    xT[ks * 128:(ks + 1) * 128, J * CHUNK:(J + 1) * CHUNK])
                x_sb.append(xt)
            for w_sb, bcol, dest in ((wq_sb, 0, qT), (wk_sb, 1, kT),
                                     (wv_sb, 2, vT)):
                ps = p1p.tile([128, CHUNK], F32, tag="proj")
                for half in range(2):
                    lo = half * 512
                    for ks in range(4):
                        nc.tensor.matmul(
                            ps[:, half * 512:(half + 1) * 512],
                            w_sb[:, ks * 128:(ks + 1) * 128],
                            x_sb[ks][:, lo:lo + 512],
                            start=(ks == 0), stop=(ks == 3),
                        )
                nc.vector.tensor_scalar_add(
                    dest[:, J * CHUNK:(J + 1) * CHUNK], ps[:],
                    bqkv_sb[:, bcol:bcol + 1])

        # V -> k-major blocks; head0/head1 transposes issued adjacently so
        # they run concurrently on PE row-groups 0-63 / 64-127
        for g in range(4):  # groups of 8 key-blocks
            tr0 = p1p.tile([128, 512], BF16, tag="vtr0")
            tr1 = p1p.tile([128, 512], BF16, tag="vtr1")
            for i in range(8):
                kb = g * 8 + i
                for hh, tr in ((0, tr0), (1, tr1)):
                    nc.tensor.transpose(
                        tr[:, i * 64:(i + 1) * 64],
                        vT[hh * 64:(hh + 1) * 64, kb * KBLK:(kb + 1) * KBLK],
                        id2_sb[hh * 64:(hh + 1) * 64, :],
                    )
            for tr, vdst in ((tr0, v0), (tr1, v1)):
                dst = vdst[:, g * 8 * 65:(g + 1) * 8 * 65]
                dst = dst.rearrange("p (k c) -> p k c", c=65)[:, :, 0:64]
                nc.vector.tensor_copy(
                    dst, tr.rearrange("p (k c) -> p k c", c=64))
        for vdst in (v0, v1):
            ones_col = vdst.rearrange("p (k c) -> p k c", c=65)[:, :, 64:65]
            nc.vector.tensor_copy(
                ones_col, onesf_sb[:].to_broadcast((128, 32, 1)))

    # ---- phase 2: attention, heads interleaved so the K=64 QK matmuls of
    # head0/head1 run concurrently on PE row-groups 0-63 / 64-127 ----
    with tc.tile_pool(name="etp", bufs=6) as etp, \
         tc.tile_pool(name="ps_st", bufs=1, space="PSUM") as ps_st, \
         tc.tile_pool(name="ps_pv", bufs=1, space="PSUM") as ps_pv:
        for J in range(NCHUNK):
            pv0 = ps_pv.tile([65, CHUNK], F32, tag="pv0")
            pv1 = ps_pv.tile([65, CHUNK], F32, tag="pv1")
            nkb = KB_PER_CHUNK * (J + 1)
            for kb in range(nkb):
                p = kb - KB_PER_CHUNK * J
                col0 = KBLK * p if p >= 0 else 0
                pieces = _pieces(col0)
                st0 = ps_st.tile([128, CHUNK], F32, tag="st0")
                st1 = ps_st.tile([128, CHUNK], F32, tag="st1")
                for st, hh in ((st0, 0), (st1, 1)):
                    hsl = slice(hh * 64, (hh + 1) * 64)
                    for (a, b) in pieces:
                        nc.tensor.matmul(
                            st[:, a:b],
                            kT[hsl, kb * KBLK:(kb + 1) * KBLK],
                            qT[hsl, J * CHUNK + a:J * CHUNK + b],
                            start=True, stop=True,
                        )
                ets = []
                for st in (st0, st1):
                    et = etp.tile([128, CHUNK], BF16, tag="et")
                    nc.scalar.activation(
                        et[:, col0:], st[:, col0:], EXP,
                        bias=kbias_sb[:, kb:kb + 1], scale=0.125)
                    if p >= 0:
                        nc.vector.tensor_mul(
                            et[:, col0:col0 + KBLK], et[:, col0:col0 + KBLK],
                            tri_sb[:])
                    ets.append(et)
                for et, vsb, pv in ((ets[0], v0, pv0), (ets[1], v1, pv1)):
                    for (a, b) in pieces:
                        # stop exactly on the last matmul touching each
                        # 512-wide psum bank region
                        last_a = (kb == KB_PER_CHUNK * J + 3 and a < 512)
                        last_b = (kb == nkb - 1)
                        nc.tensor.matmul(
                            pv[:, a:b],
                            vsb[:, kb * 65:(kb + 1) * 65],
                            et[:, a:b],
                            start=(kb == 0),
                            stop=(last_a if a < 512 else last_b),
                        )
            csl = slice(J * CHUNK, (J + 1) * CHUNK)
            nc.vector.tensor_copy(oT01[0:64, csl], pv0[0:64, :])
            nc.vector.tensor_copy(oT01[64:128, csl], pv1[0:64, :])
            nc.vector.tensor_copy(den0[:, csl], pv0[64:65, :])
            nc.vector.tensor_copy(den1[:, csl], pv1[64:65, :])

    # ---- phase 3: per-head output projection (undivided), heads row-paired ----
    nc.sync.dma_start(dens[0:1, :], den0[:])
    nc.sync.dma_start(dens[1:2, :], den1[:])
    with tc.tile_pool(name="ph3sb", bufs=2) as p3s, \
         tc.tile_pool(name="ps_po", bufs=2, space="PSUM") as ps_po:
        for J in range(NCHUNK):
            csl = slice(J * CHUNK, (J + 1) * CHUNK)
            for dt_ in range(4):
                po0 = ps_po.tile([128, CHUNK], F32, tag="po0")
                po1 = ps_po.tile([128, CHUNK], F32, tag="po1")
                for (a, b) in _pieces(0):
                    for po, hh in ((po0, 0), (po1, 1)):
                        hsl = slice(hh * 64, (hh + 1) * 64)
                        nc.tensor.matmul(
                            po[:, a:b],
                            wo01_sb[hsl, dt_ * 128:(dt_ + 1) * 128],
                            oT01[hsl, J * CHUNK + a:J * CHUNK + b],
                            start=True, stop=True)
                for po, poT, cpy in ((po0, po0T, nc.vector.tensor_copy),
                                     (po1, po1T, nc.scalar.copy)):
                    posb = p3s.tile([128, CHUNK], F32, tag="posb", bufs=4)
                    cpy(posb[:], po[:])
                    nc.sync.dma_start(poT[dt_ * 128:(dt_ + 1) * 128, csl],
                                      posb[:])


_CACHED = None


def _build():
    global _CACHED
    if _CACHED is not None:
        return _CACHED
    nc = bacc.Bacc("TRN2", target_bir_lowering=False, debug=False,
                   enable_asserts=False, num_devices=NCORES)
    names = [
        ("xT", [D, S], BF16), ("wq_p", [128, 512], BF16),
        ("wk_p", [128, 512], BF16), ("wv_p", [128, 512], BF16),
        ("wo01", [128, 512], BF16),
        ("bqkv", [128, 3], F32), ("kbias", [128, 32], F32),
        ("trimask", [128, 128], BF16), ("ident2", [128, 64], BF16),
    ]
    aps = [nc.dram_tensor(n, sh, dt_, kind="ExternalInput").ap()
           for n, sh, dt_ in names]
    po0T = nc.dram_tensor("po0T", [D, S], F32, kind="ExternalOutput").ap()
    po1T = nc.dram_tensor("po1T", [D, S], F32, kind="ExternalOutput").ap()
    dens = nc.dram_tensor("dens", [2, S], F32, kind="ExternalOutput").ap()
    with tile.TileContext(nc) as tc, ExitStack() as ctx:
        _emit(nc, tc, ctx, aps + [po0T, po1T, dens])
    nc.compile()
    _CACHED = nc
    return nc


def _host_inputs(x, attention_mask, Wq, bq, Wk, bk, Wv, bv, Wo, bo):
    f = np.float32
    x = np.asarray(x, f)
    mask = np.asarray(attention_mask)
    Wq, Wk, Wv, Wo = (np.asarray(w, f) for w in (Wq, Wk, Wv, Wo))
    bq, bk, bv = (np.asarray(b_, f) for b_ in (bq, bk, bv))
    tri = np.triu(np.ones((128, 128), NPBF16))      # [k,q]: 1 where q >= k
    id2 = np.tile(np.eye(64, dtype=NPBF16), (2, 1))
    in_maps = []
    for c in range(NCORES):
        b = c // 4
        h0 = 2 * (c % 4)
        hsl = slice(64 * h0, 64 * h0 + 128)

        def pack_w(W):
            wt = W[hsl, :].T                        # [512, 128] = Wh^T
            return np.ascontiguousarray(
                wt.reshape(4, 128, 128).transpose(1, 0, 2)
                .reshape(128, 512).astype(NPBF16))

        wo_t = Wo[:, hsl].T.astype(NPBF16)           # [128, 512]
        kb = np.where(mask[b] != 0, f(0.0), f(NEG)).astype(f)
        in_maps.append({
            "xT": np.ascontiguousarray(x[b].T.astype(NPBF16)),
            "wq_p": pack_w(Wq), "wk_p": pack_w(Wk), "wv_p": pack_w(Wv),
            "wo01": np.ascontiguousarray(wo_t),
            "bqkv": np.ascontiguousarray(
                np.stack([bq[hsl], bk[hsl], bv[hsl]], axis=1)),
            "kbias": np.ascontiguousarray(kb.reshape(32, 128).T),
            "trimask": tri, "ident2": id2,
        })
    return in_maps


def _assemble(results, bo):
    out = np.zeros((B, S, D), np.float32)
    for c in range(NCORES):
        r = results[c]
        dens = r["dens"]
        part = r["po0T"] / dens[0:1, :] + r["po1T"] / dens[1:2, :]
        out[c // 4] += part.T
    out += np.asarray(bo, np.float32)
    return out


def kernel(**inputs) -> np.ndarray:
    nc = _build()
    in_maps = _host_inputs(**inputs)
    last_err = None
    for attempt in range(3):
        try:
            res = bass_utils.run_bass_kernel_spmd(
                nc, in_maps, core_ids=list(range(NCORES)))
            out = _assemble(res.results, inputs["bo"])
        except Exception as e:  # transient NRT/axon device errors
            last_err = e
            continue
        if np.isfinite(out).all():
            return out
        last_err = RuntimeError("non-finite output")
    raise last_err


def run_traced(inputs, **kwargs):
    """test.py helper: run with NTFF tracing, return (out, BassKernelResults)."""
    nc = _build()
    in_maps = _host_inputs(**inputs)
    res = bass_utils.run_bass_kernel_spmd(
        nc, in_maps, core_ids=list(range(NCORES)), trace=True, **kwargs)
    return _assemble(res.results, inputs["bo"]), res


# revision 18
# speedup vs baseline: 1.2436x; 1.0430x over previous
"""Causal self-attention (B=2, S=4096, D=512, H=8) on 8 Trainium2 cores.

Sharding: core c handles batch b = c//4 and heads {2*(c%4), 2*(c%4)+1}.
Each core computes q/k/v projections for its two heads, causal flash-style
attention in a transposed (k-major) score layout, and per-head undivided
output-projection partials po_h^T = Wo_h @ attn_h^T plus the softmax
denominators.  The host divides by the denominators, sums the 4 cores per
batch, adds bo, and transposes back.

Device layout notes:
  qT/kT: [128, S] bf16, rows 0-63 head0, 64-127 head1 (head dim on
  partitions).  scores^T tiles: [128 keys, 1024 queries]; exp on ACT reads
  PSUM directly with the padding mask folded into the per-partition bias
  and 1/sqrt(hd) into the scale, writing bf16.  V is PE-transposed to
  k-major [128, 65]-blocks with a ones column appended, so the PV matmul
  accumulates numerators and the softmax denominator (row 64) together.
  All matmul operands are bf16 (full-rate PE + fast weight load);
  accumulation stays fp32 in PSUM.
"""

import sys

sys.path.insert(0, "/opt/trn_rl_repo")

from contextlib import ExitStack

import ml_dtypes
import numpy as np

import concourse.bass as bass
import concourse.tile as tile
from concourse import bacc, bass_utils, mybir

B, S, D = 2, 4096, 512
H, HD = 8, 64
NCORES = 8
F32 = mybir.dt.float32
BF16 = mybir.dt.bfloat16
EXP = mybir.ActivationFunctionType.Exp
NPBF16 = ml_dtypes.bfloat16

CHUNK = 1024                  # query-chunk width
NCHUNK = S // CHUNK           # 4
KBLK = 128                    # key block (partition dim)
KB_PER_CHUNK = CHUNK // KBLK  # 8
NEG = -1.0e30


def _pieces(col0):
    """Split [col0, CHUNK) into <=512-wide pieces aligned to 512 boundaries."""
    out = []
    c = col0
    while c < CHUNK:
        nxt = min(CHUNK, (c // 512 + 1) * 512)
        out.append((c, nxt))
        c = nxt
    return out


def _emit(nc, tc, ctx, io):
    xT, wq_p, wk_p, wv_p, wo01d, bqkv, kbias, trimask, ident2, \
        po0T, po1T, dens = io

    const = ctx.enter_context(tc.tile_pool(name="const", bufs=1))
    poolA = ctx.enter_context(tc.tile_pool(name="poolA", bufs=1))
    poolB = ctx.enter_context(tc.tile_pool(name="poolB", bufs=1))

    # ---- constants / weights into SBUF ----
    wq_sb = const.tile([128, 512], BF16, tag="wq")
    wk_sb = const.tile([128, 512], BF16, tag="wk")
    wv_sb = const.tile([128, 512], BF16, tag="wv")
    wo01_sb = const.tile([128, 512], BF16, tag="wo01")
    bqkv_sb = const.tile([128, 3], F32, tag="bqkv")
    kbias_sb = const.tile([128, 32], F32, tag="kbias")
    tri_sb = const.tile([128, 128], BF16, tag="tri")
    id2_sb = const.tile([128, 64], BF16, tag="id2")
    onesf_sb = const.tile([128, 1], F32, tag="onesf")
    nc.vector.memset(onesf_sb[:], 1.0)
    for t, a in ((wq_sb, wq_p), (wk_sb, wk_p), (wv_sb, wv_p),
                 (bqkv_sb, bqkv), (id2_sb, ident2), (kbias_sb, kbias),
                 (tri_sb, trimask), (wo01_sb, wo01d)):
        nc.sync.dma_start(t[:], a[:])

    # ---- intermediates: poolA spans phases 1-2, poolB phases 2-3 ----
    qT = poolA.tile([128, S], BF16, tag="qT")
    kT = poolA.tile([128, S], BF16, tag="kT")
    v0 = poolA.tile([128, 32 * 65], BF16, tag="v0")
    v1 = poolA.tile([128, 32 * 65], BF16, tag="v1")
    oT01 = poolB.tile([128, S], BF16, tag="oT01")
    den0 = poolB.tile([1, S], F32, tag="den0")
    den1 = poolB.tile([1, S], F32, tag="den1")

    # ---- phase 1: q/k/v projections (+ v transpose to k-major) ----
    with tc.tile_pool(name="ph1sb", bufs=1) as p1s, \
         tc.tile_pool(name="ph1ps", bufs=2, space="PSUM") as p1p:
        vT = p1s.tile([128, S], BF16, tag="vT")

        for J in range(NCHUNK):
            x_sb = []
            for ks in range(4):
                xt = p1s.tile([128, CHUNK], BF16, tag=f"x{ks}", bufs=2)
                nc.sync.dma_start(
                    xt[:],
                    xT[ks * 128:(ks + 1) * 128, J * CHUNK:(J + 1) * CHUNK])
                x_sb.append(xt)
            for w_sb, bcol, dest in ((wq_sb, 0, qT), (wk_sb, 1, kT),
                                     (wv_sb, 2, vT)):
                ps = p1p.tile([128, CHUNK], F32, tag="proj")
                for half in range(2):
                    lo = half * 512
                    for ks in range(4):
                        nc.tensor.matmul(
                            ps[:, half * 512:(half + 1) * 512],
                            w_sb[:, ks * 128:(ks + 1) * 128],
                            x_sb[ks][:, lo:lo + 512],
                            start=(ks == 0), stop=(ks == 3),
                        )
                nc.vector.tensor_scalar_add(
                    dest[:, J * CHUNK:(J + 1) * CHUNK], ps[:],
                    bqkv_sb[:, bcol:bcol + 1])

        # V -> k-major blocks; head0/head1 transposes issued adjacently so
        # they run concurrently on PE row-groups 0-63 / 64-127
        for g in range(4):  # groups of 8 key-blocks
            tr0 = p1p.tile([128, 512], BF16, tag="vtr0")
            tr1 = p1p.tile([128, 512], BF16, tag="vtr1")
            for i in range(8):
                kb = g * 8 + i
                for hh, tr in ((0, tr0), (1, tr1)):
                    nc.tensor.transpose(
                        tr[:, i * 64:(i + 1) * 64],
                        vT[hh * 64:(hh + 1) * 64, kb * KBLK:(kb + 1) * KBLK],
                        id2_sb[hh * 64:(hh + 1) * 64, :],
                    )
            for tr, vdst in ((tr0, v0), (tr1, v1)):
                dst = vdst[:, g * 8 * 65:(g + 1) * 8 * 65]
                dst = dst.rearrange("p (k c) -> p k c", c=65)[:, :, 0:64]
                nc.vector.tensor_copy(
                    dst, tr.rearrange("p (k c) -> p k c", c=64))
        for vdst in (v0, v1):
            ones_col = vdst.rearrange("p (k c) -> p k c", c=65)[:, :, 64:65]
            nc.vector.tensor_copy(
                ones_col, onesf_sb[:].to_broadcast((128, 32, 1)))

    # ---- phase 2: attention, heads interleaved so the K=64 QK matmuls of
    # head0/head1 run concurrently on PE row-groups 0-63 / 64-127 ----
    with tc.tile_pool(name="etp", bufs=8) as etp, \
         tc.tile_pool(name="ps_st", bufs=1, space="PSUM") as ps_st, \
         tc.tile_pool(name="ps_pv", bufs=1, space="PSUM") as ps_pv:
        for J in range(NCHUNK):
            pv0 = ps_pv.tile([65, CHUNK], F32, tag="pv0")
            pv1 = ps_pv.tile([65, CHUNK], F32, tag="pv1")
            nkb = KB_PER_CHUNK * (J + 1)
            for kb in range(nkb):
                p = kb - KB_PER_CHUNK * J
                col0 = KBLK * p if p >= 0 else 0
                pieces = _pieces(col0)
                st0 = ps_st.tile([128, CHUNK], F32, tag="st0")
                st1 = ps_st.tile([128, CHUNK], F32, tag="st1")
                for st, hh in ((st0, 0), (st1, 1)):
                    hsl = slice(hh * 64, (hh + 1) * 64)
                    for (a, b) in pieces:
                        nc.tensor.matmul(
                            st[:, a:b],
                            kT[hsl, kb * KBLK:(kb + 1) * KBLK],
                            qT[hsl, J * CHUNK + a:J * CHUNK + b],
                            start=True, stop=True,
                        )
                ets = []
                for st in (st0, st1):
                    et = etp.tile([128, CHUNK], BF16, tag="et")
                    nc.scalar.activation(
                        et[:, col0:], st[:, col0:], EXP,
                        bias=kbias_sb[:, kb:kb + 1], scale=0.125)
                    if p >= 0:
                        nc.vector.tensor_mul(
                            et[:, col0:col0 + KBLK], et[:, col0:col0 + KBLK],
                            tri_sb[:])
                    ets.append(et)
                for et, vsb, pv in ((ets[0], v0, pv0), (ets[1], v1, pv1)):
                    for (a, b) in pieces:
                        # stop exactly on the last matmul touching each
                        # 512-wide psum bank region
                        last_a = (kb == KB_PER_CHUNK * J + 3 and a < 512)
                        last_b = (kb == nkb - 1)
                        nc.tensor.matmul(
                            pv[:, a:b],
                            vsb[:, kb * 65:(kb + 1) * 65],
                            et[:, a:b],
                            start=(kb == 0),
                            stop=(last_a if a < 512 else last_b),
                        )
            csl = slice(J * CHUNK, (J + 1) * CHUNK)
            nc.vector.tensor_copy(oT01[0:64, csl], pv0[0:64, :])
            nc.vector.tensor_copy(oT01[64:128, csl], pv1[0:64, :])
            nc.vector.tensor_copy(den0[:, csl], pv0[64:65, :])
            nc.vector.tensor_copy(den1[:, csl], pv1[64:65, :])

    # ---- phase 3: per-head output projection (undivided), heads row-paired ----
    nc.sync.dma_start(dens[0:1, :], den0[:])
    nc.sync.dma_start(dens[1:2, :], den1[:])
    with tc.tile_pool(name="ph3sb", bufs=2) as p3s, \
         tc.tile_pool(name="ps_po", bufs=2, space="PSUM") as ps_po:
        for J in range(NCHUNK):
            csl = slice(J * CHUNK, (J + 1) * CHUNK)
            for dt_ in range(4):
                po0 = ps_po.tile([128, CHUNK], F32, tag="po0")
                po1 = ps_po.tile([128, CHUNK], F32, tag="po1")
                for (a, b) in _pieces(0):
                    for po, hh in ((po0, 0), (po1, 1)):
                        hsl = slice(hh * 64, (hh + 1) * 64)
                        nc.tensor.matmul(
                            po[:, a:b],
                            wo01_sb[hsl, dt_ * 128:(dt_ + 1) * 128],
                            oT01[hsl, J * CHUNK + a:J * CHUNK + b],
                            start=True, stop=True)
                for po, poT, cpy in ((po0, po0T, nc.vector.tensor_copy),
                                     (po1, po1T, nc.scalar.copy)):
                    posb = p3s.tile([128, CHUNK], F32, tag="posb", bufs=4)
                    cpy(posb[:], po[:])
                    nc.sync.dma_start(poT[dt_ * 128:(dt_ + 1) * 128, csl],
                                      posb[:])


_CACHED = None


def _build():
    global _CACHED
    if _CACHED is not None:
        return _CACHED
    nc = bacc.Bacc("TRN2", target_bir_lowering=False, debug=False,
                   enable_asserts=False, num_devices=NCORES)
    names = [
        ("xT", [D, S], BF16), ("wq_p", [128, 512], BF16),
        ("wk_p", [128, 512], BF16), ("wv_p", [128, 512], BF16),
        ("wo01", [128, 512], BF16),
        ("bqkv", [128, 3], F32), ("kbias", [128, 32], F32),
        ("trimask", [128, 128], BF16), ("ident2", [128, 64], BF16),
    ]
    aps = [nc.dram_tensor(n, sh, dt_, kind="ExternalInput").ap()
           for n, sh, dt_ in names]
    po0T = nc.dram_tensor("po0T", [D, S], F32, kind="ExternalOutput").ap()
    po1T = nc.dram_tensor("po1T", [D, S], F32, kind="ExternalOutput").ap()
    dens = nc.dram_tensor("dens", [2, S], F32, kind="ExternalOutput").ap()
    with tile.TileContext(nc) as tc, ExitStack() as ctx:
        _emit(nc, tc, ctx, aps + [po0T, po1T, dens])
    nc.compile()
    _CACHED = nc
    return nc


def _host_inputs(x, attention_mask, Wq, bq, Wk, bk, Wv, bv, Wo, bo):
    f = np.float32
    x = np.asarray(x, f)
    mask = np.asarray(attention_mask)
    Wq, Wk, Wv, Wo = (np.asarray(w, f) for w in (Wq, Wk, Wv, Wo))
    bq, bk, bv = (np.asarray(b_, f) for b_ in (bq, bk, bv))
    tri = np.triu(np.ones((128, 128), NPBF16))      # [k,q]: 1 where q >= k
    id2 = np.tile(np.eye(64, dtype=NPBF16), (2, 1))
    in_maps = []
    for c in range(NCORES):
        b = c // 4
        h0 = 2 * (c % 4)
        hsl = slice(64 * h0, 64 * h0 + 128)

        def pack_w(W):
            wt = W[hsl, :].T                        # [512, 128] = Wh^T
            return np.ascontiguousarray(
                wt.reshape(4, 128, 128).transpose(1, 0, 2)
                .reshape(128, 512).astype(NPBF16))

        wo_t = Wo[:, hsl].T.astype(NPBF16)           # [128, 512]
        kb = np.where(mask[b] != 0, f(0.0), f(NEG)).astype(f)
        in_maps.append({
            "xT": np.ascontiguousarray(x[b].T.astype(NPBF16)),
            "wq_p": pack_w(Wq), "wk_p": pack_w(Wk), "wv_p": pack_w(Wv),
            "wo01": np.ascontiguousarray(wo_t),
            "bqkv": np.ascontiguousarray(
                np.stack([bq[hsl], bk[hsl], bv[hsl]], axis=1)),
            "kbias": np.ascontiguousarray(kb.reshape(32, 128).T),
            "trimask": tri, "ident2": id2,
        })
    return in_maps


def _assemble(results, bo):
    out = np.zeros((B, S, D), np.float32)
    for c in range(NCORES):
        r = results[c]
        dens = r["dens"]
        part = r["po0T"] / dens[0:1, :] + r["po1T"] / dens[1:2, :]
        out[c // 4] += part.T
    out += np.asarray(bo, np.float32)
    return out


def kernel(**inputs) -> np.ndarray:
    nc = _build()
    in_maps = _host_inputs(**inputs)
    last_err = None
    for attempt in range(3):
        try:
            res = bass_utils.run_bass_kernel_spmd(
                nc, in_maps, core_ids=list(range(NCORES)))
            out = _assemble(res.results, inputs["bo"])
        except Exception as e:  # transient NRT/axon device errors
            last_err = e
            continue
        if np.isfinite(out).all():
            return out
        last_err = RuntimeError("non-finite output")
    raise last_err


def run_traced(inputs, **kwargs):
    """test.py helper: run with NTFF tracing, return (out, BassKernelResults)."""
    nc = _build()
    in_maps = _host_inputs(**inputs)
    res = bass_utils.run_bass_kernel_spmd(
        nc, in_maps, core_ids=list(range(NCORES)), trace=True, **kwargs)
    return _assemble(res.results, inputs["bo"]), res
